# revision 51
# baseline (speedup 1.0000x reference)
"""Causal self-attention (T=4096, D=1024, H=16) on 8 TRN2 NeuronCores.

Sharding: tensor-parallel over heads. Core i owns heads (2i, 2i+1):
  - computes its 384-row slice of the QKV projection (bf16 matmuls),
  - causal attention for its 2 heads in transposed orientation
    (scores sT[tk, tq] so the AV contraction needs no transposes),
  - scores for the two heads run CONCURRENTLY on the PE via row-group
    tiling (each head contracts over 64 partitions -> disjoint row
    groups (0,0)/(64,0)),
  - causal mask applied on GPSIMD (affine_select zero-fills the masked
    triangle of the exp'd tile); fully-masked column prefixes of
    diagonal tiles are simply never computed (trimmed extents in the
    score matmuls, the exp, and the AV matmuls),
  - softmax denominators via a ones-column appended to V,
  - its 128-dim slice of the c_proj contraction -> partial output [1024, 4096].
Host sums the 8 partial outputs (the "all-reduce"), transposes, adds b_proj.
"""

import math

import ml_dtypes
import numpy as np

import concourse.bass as bass
import concourse.mybir as mybir
import concourse.tile as tile
from concourse import bacc
from concourse.bass import ts
from concourse.bass_utils import run_bass_kernel_spmd
from concourse.masks import make_identity

F32 = mybir.dt.float32
F32R = mybir.dt.float32r
BF16 = mybir.dt.bfloat16
Exp = mybir.ActivationFunctionType.Exp

T = 4096
DM = 1024
NCORES = 8
NW = 8          # tq windows of 512
TQW = 512
NKT = 32        # tk tiles of 128
CT = 8          # c (d_model) tiles of 128


def build_nc():
    nc = bacc.Bacc(None, target_bir_lowering=False)

    xT = nc.dram_tensor("xT", [DM, T], BF16, kind="ExternalInput")
    wqkvT = nc.dram_tensor("wqkvT", [DM, 384], BF16, kind="ExternalInput")
    bqkv = nc.dram_tensor("bqkv", [128, 3], F32, kind="ExternalInput")
    wpT = nc.dram_tensor("wpT", [128, DM], BF16, kind="ExternalInput")
    out = nc.dram_tensor("out", [DM, T], F32, kind="ExternalOutput")

    xT_r = xT.ap().rearrange("(ct p) t -> p ct t", p=128)
    wq_r = wqkvT.ap().rearrange("(ct p) j -> p ct j", p=128)

    with tile.TileContext(nc) as tc:
        with (
            tc.tile_pool(name="const", bufs=1) as const,
            tc.tile_pool(name="xw", bufs=4) as xw_pool,
            tc.tile_pool(name="vtmp", bufs=2) as vtmp_pool,
            tc.tile_pool(name="pt", bufs=8) as pt_pool,
            tc.tile_pool(name="ycp", bufs=4) as ycp_pool,
            tc.tile_pool(name="rb", bufs=2) as rb_pool,
            tc.tile_pool(name="yn", bufs=3) as yn_pool,
            tc.tile_pool(name="ynhi", bufs=2) as ynhi_pool,
            tc.tile_pool(name="ob", bufs=4) as ob_pool,
            tc.tile_pool(name="rsp", bufs=2) as rsp_pool,
            tc.tile_pool(name="rec", bufs=1) as rec_pool,
            tc.tile_pool(name="ps_sc", bufs=2, space="PSUM") as ps_sc,
            tc.tile_pool(name="ps_av", bufs=2, space="PSUM") as ps_av,
            tc.tile_pool(name="ps_fl", bufs=2, space="PSUM") as ps_fl,
            tc.tile_pool(name="rdram", bufs=2, space="DRAM") as dram_pool,
        ):
            # ---- constants. Prologue inputs are split across the SP and
            # Activation hardware DGE queues (the scalar engine is idle
            # until the first real exp) to halve the serial input phase.
            wq_sb = const.tile([128, CT, 384], BF16)
            xw0 = xw_pool.tile([128, CT, TQW], BF16, tag="xw")
            # per-ct interleaved chunks across both DGE queues: the ct-th
            # QKV matmul of the first chain only waits for its own slices
            for ct in range(CT):
                nc.scalar.dma_start(
                    out=wq_sb[:, ct, ts(0, 128)], in_=wq_r[:, ct, ts(0, 128)]
                )
                nc.sync.dma_start(out=xw0[:, ct, :], in_=xT_r[:, ct, ts(0, TQW)])
            xw_tiles0 = xw0
            nc.scalar.dma_start(
                out=wq_sb[:, :, ts(1, 128)], in_=wq_r[:, :, ts(1, 128)]
            )
            nc.scalar.dma_start(
                out=wq_sb[:, :, ts(2, 128)], in_=wq_r[:, :, ts(2, 128)]
            )
            wp_sb = const.tile([128, DM], BF16)
            nc.sync.dma_start(out=wp_sb[:], in_=wpT[:])
            bq_sb = const.tile([128, 3], F32)
            nc.scalar.dma_start(out=bq_sb[:], in_=bqkv[:])
            ident = const.tile([128, 128], BF16)
            make_identity(nc, ident[:])
            ones1 = const.tile([1, 64], F32)
            nc.vector.memset(ones1[:], 1.0)
            qT = const.tile([128, T], BF16)
            kT = const.tile([128, T], BF16)
            v_aug = const.tile([128, NKT, 130], BF16)
            # only the two ones-columns need initializing (the v regions are
            # overwritten by the transposes); a full-tile memset costs 3.5us
            # at the head of the DVE queue and delays the first bias-adds
            nc.vector.memset(v_aug[:, :, 64:65], 1.0)
            nc.vector.memset(v_aug[:, :, 129:130], 1.0)

            # warm-up: preload the EXP table off the critical path and keep
            # the PE busy through the initial input-DMA wait so the HAM
            # clock gate releases before the real work starts
            scr = const.tile([128, 16], BF16)
            nc.scalar.activation(out=scr[:], in_=ident[:, 0:16], func=Exp)
            wps = ps_fl.tile([128, 128], F32, tag="fl")

            def pe_keepwarm(n):
                # dependency-free matmuls into a dedicated psum tile: fill
                # PE idle windows so the HAM clock gate never re-throttles
                for _ in range(n):
                    nc.tensor.matmul(
                        wps[:], ident[:], ident[:], start=True, stop=True
                    )

            pe_keepwarm(80)

            # ---- filler units (QKV projection + output projection) ----
            def f_xw_dma(w, split=False):
                def go():
                    if w in xw_tiles:
                        return
                    xw = xw_pool.tile([128, CT, TQW], BF16, tag="xw")
                    for ct in range(CT):
                        eng = nc.scalar if (split and ct % 2) else nc.sync
                        eng.dma_start(out=xw[:, ct, :], in_=xT_r[:, ct, ts(w, TQW)])
                    xw_tiles[w] = xw
                return go

            def f_qkv_rb(w, rb, pool=None, tag="fl"):
                def go():
                    xw = xw_tiles[w]
                    ps = (pool or ps_fl).tile([128, TQW], F32, tag=tag)
                    for ct in range(CT):
                        nc.tensor.matmul(
                            ps[:],
                            wq_sb[:, ct, ts(rb, 128)],
                            xw[:, ct, :],
                            start=(ct == 0),
                            stop=(ct == CT - 1),
                        )
                    if rb == 0:
                        nc.vector.tensor_scalar_add(
                            out=qT[:, ts(w, TQW)], in0=ps[:], scalar1=bq_sb[:, 0:1]
                        )
                    elif rb == 1:
                        nc.vector.tensor_scalar_add(
                            out=kT[:, ts(w, TQW)], in0=ps[:], scalar1=bq_sb[:, 1:2]
                        )
                    else:
                        vt = vtmp_pool.tile([128, TQW], BF16, tag="vt")
                        nc.vector.tensor_scalar_add(
                            out=vt[:], in0=ps[:], scalar1=bq_sb[:, 2:3]
                        )
                        vt_tiles[w] = vt
                return go

            def f_vtrans(w, k):
                def go():
                    vt = vt_tiles[w]
                    i = 4 * w + k
                    pst = ps_fl.tile([128, 128], BF16, tag="fl")
                    nc.tensor.transpose(pst[:], vt[:, ts(k, 128)], ident[:])
                    nc.vector.tensor_copy(out=v_aug[:, i, 0:64], in_=pst[:, 0:64])
                    nc.vector.tensor_copy(out=v_aug[:, i, 65:129], in_=pst[:, 64:128])
                return go

            def f_proj(j, ot, pool=None, tag="fl"):
                def go():
                    yn = yn_tiles[j]
                    pp = (pool or ps_fl).tile([128, TQW], F32, tag=tag)
                    nc.tensor.matmul(
                        pp[:], wp_sb[:, ts(ot, 128)], yn[:], start=True, stop=True
                    )
                    ob = ob_pool.tile([128, TQW], F32, tag="ob")
                    nc.vector.tensor_copy(out=ob[:], in_=pp[:])
                    nc.sync.dma_start(out=out[ts(ot, 128), ts(j, TQW)], in_=ob[:])
                return go

            xw_tiles = {}
            vt_tiles = {}
            yn_tiles = {}

            pts = {}

            def emit_tile(j, i):
                """Scores (row-tiled concurrent pair) + exp + mask for
                k-tile i of window j; leaves pt in pts[(j, i)]."""
                k = i - 4 * j
                diag = k >= 0
                lo = 128 * k if diag else 0
                sp = ps_sc.tile([128, 2, TQW], F32, tag="sc")
                nc.tensor.matmul(
                    sp[:, 0, lo:TQW],
                    kT[0:64, ts(i, 128)],
                    qT[0:64, 512 * j + lo : 512 * (j + 1)],
                    start=True,
                    stop=True,
                )
                nc.tensor.matmul(
                    sp[:, 1, lo:TQW],
                    kT[64:128, ts(i, 128)],
                    qT[64:128, 512 * j + lo : 512 * (j + 1)],
                    start=True,
                    stop=True,
                )
                pt = pt_pool.tile([128, 2, TQW], BF16, tag="pt")
                nc.scalar.activation(
                    out=pt[:, :, lo:TQW], in_=sp[:, :, lo:TQW], func=Exp
                )
                if diag:
                    # zero the masked triangle: keep iff (c - r) >= 0
                    nc.gpsimd.affine_select(
                        out=pt[:, :, lo : lo + 128],
                        in_=pt[:, :, lo : lo + 128],
                        pattern=[[0, 2], [1, 128]],
                        compare_op=mybir.AluOpType.is_ge,
                        fill=0.0,
                        base=0,
                        channel_multiplier=-1,
                    )
                pts[(j, i)] = pt

            def emit_attn(j, fillers, prev_tail=None, head_done=False):
                """Window j: scores+exp+mask+AV loop with fillers interleaved.

                prev_tail (engine-only, no PE instructions) is emitted at
                iteration 0 so the previous window's normalize chain starts
                as early as possible. PE fillers wait until iteration 2 —
                they may depend on fresh DMAs, and a blocked filler blocks
                everything behind it in PE program order. If head_done, the
                first two tiles were already emitted inside the previous
                window (boundary software pipelining).
                """
                ntk = 4 * (j + 1)
                yh0 = ps_av.tile([65, TQW], F32, tag="av")
                yh1 = ps_av.tile([65, TQW], F32, tag="av")
                nf = len(fillers)
                fi = 0
                span = max(1, ntk - 2)

                def emit_av(i):
                    k = i - 4 * j
                    lo = 128 * k if k >= 0 else 0
                    pt = pts[(j, i)]
                    nc.tensor.matmul(
                        yh0[:, lo:TQW],
                        v_aug[:, i, 0:65],
                        pt[:, 0, lo:TQW],
                        start=(i == 0),
                        stop=(i == ntk - 1),
                    )
                    nc.tensor.matmul(
                        yh1[:, lo:TQW],
                        v_aug[:, i, 65:130],
                        pt[:, 1, lo:TQW],
                        start=(i == 0),
                        stop=(i == ntk - 1),
                    )
                    del pts[(j, i)]

                for i in range(ntk):
                    if i == 0 and prev_tail is not None:
                        prev_tail()
                    # interleave filler units evenly across iterations 2..ntk
                    while fi < nf * max(0, i - 1) // span:
                        fillers[fi]()
                        fi += 1
                    if not (head_done and i < 2):
                        emit_tile(j, i)
                    if i >= 2:
                        emit_av(i - 2)
                # boundary software pipelining: the next window's first two
                # score+exp tiles slot in around the final AV chains so
                # neither the PE nor the ACT idles across the boundary
                half = fi + (nf - fi + 1) // 2
                if j + 1 < NW:
                    emit_tile(j + 1, 0)
                emit_av(ntk - 2)
                while fi < half:
                    fillers[fi]()
                    fi += 1
                if j + 1 < NW:
                    emit_tile(j + 1, 1)
                emit_av(ntk - 1)
                while fi < nf:
                    fillers[fi]()
                    fi += 1

                # Tail: copy yh psum -> sbuf right away (releases the psum
                # banks for the next window's AV), then denominators ->
                # reciprocal -> broadcast -> norm-muls from the sbuf copy.
                # Returned as a closure; the main loop emits it at iteration
                # 0 of the next window.
                def tail():
                    ycp0 = ycp_pool.tile([65, TQW], F32, tag="ycp")
                    ycp1 = ycp_pool.tile([65, TQW], F32, tag="ycp")
                    yn = yn_pool.tile([128, TQW], BF16, tag="yn")
                    nc.vector.tensor_copy(out=ycp0[:], in_=yh0[:])
                    nc.vector.tensor_copy(out=ycp1[:], in_=yh1[:])
                    if j == NW - 1:
                        # end of kernel: latency matters and the PE is idle —
                        # single-lane reciprocal, then broadcast across
                        # partitions with a ones-matmul (exact: x*1.0)
                        rec = rec_pool.tile([1, 1024], F32, tag="rec")
                        nc.vector.reciprocal(out=rec[:, 0:512], in_=ycp0[64:65, :])
                        nc.vector.reciprocal(
                            out=rec[:, 512:1024], in_=ycp1[64:65, :]
                        )
                        bc0 = ps_sc.tile([64, TQW], F32, tag="sc")
                        bc1 = ps_sc.tile([64, TQW], F32, tag="sc")
                        nc.tensor.matmul(
                            bc0[:], ones1[:], rec[:, 0:512], start=True, stop=True
                        )
                        nc.tensor.matmul(
                            bc1[:], ones1[:], rec[:, 512:1024], start=True, stop=True
                        )
                        nc.vector.tensor_mul(
                            out=yn[0:64, :], in0=ycp0[0:64, :], in1=bc0[:]
                        )
                        yh = ynhi_pool.tile([64, TQW], BF16, tag="ynhi")
                        nc.vector.tensor_mul(
                            out=yh[:], in0=ycp1[0:64, :], in1=bc1[:]
                        )
                        nc.sync.dma_start(out=yn[64:128, :], in_=yh[:])
                        yn_tiles[j] = yn
                        return
                    rd = dram_pool.tile([1, 1024], F32, tag="rd")
                    rb_t = rb_pool.tile([64, 1024], F32, tag="rb")
                    rsp = rsp_pool.tile([128, 2, 4], F32, tag="rsp")
                    nc.sync.dma_start(out=rsp[:, 0, :], in_=ycp0[64:65, :])
                    nc.sync.dma_start(out=rsp[:, 1, :], in_=ycp1[64:65, :])
                    nc.vector.reciprocal(out=rsp[:], in_=rsp[:])
                    nc.sync.dma_start(out=rd[:, 0:512], in_=rsp[:, 0, :])
                    nc.sync.dma_start(out=rd[:, 512:1024], in_=rsp[:, 1, :])
                    bcast_all = bass.AP(
                        tensor=rd.tensor, offset=rd.offset, ap=[[0, 64], [1, 1024]]
                    )
                    nc.sync.dma_start(out=rb_t[:], in_=bcast_all)
                    nc.vector.tensor_mul(
                        out=yn[0:64, :], in0=ycp0[0:64, :], in1=rb_t[:, 0:512]
                    )
                    yh = ynhi_pool.tile([64, TQW], BF16, tag="ynhi")
                    nc.vector.tensor_mul(
                        out=yh[:], in0=ycp1[0:64, :], in1=rb_t[:, 512:1024]
                    )
                    nc.sync.dma_start(out=yn[64:128, :], in_=yh[:])
                    yn_tiles[j] = yn

                return tail

            # ---- emission ----
            # prologue: QKV for windows 0..2 as a dense burst through the
            # scores psum slots (attn not running yet -> ungated ping-pong)
            xw_tiles[0] = xw_tiles0
            f_xw_dma(1, split=True)()
            f_xw_dma(2, split=True)()
            f_xw_dma(3, split=True)()
            for rb in range(3):
                f_qkv_rb(0, rb, pool=ps_sc, tag="sc")()
                pe_keepwarm(10)
            # window 0's first two score+exp tiles go ahead of the V
            # transposes so the ACT pipeline starts as soon as q/k land
            emit_tile(0, 0)
            pe_keepwarm(6)
            emit_tile(0, 1)
            for k in range(4):
                f_vtrans(0, k)()

            # attn(0) carries QKV 1+2; attn(j) carries QKV j+2 and proj j-2
            # (proj deferred two windows so the tail chain has a full window
            # of latency headroom before its consumers hit the PE queue)
            prev_tail = None
            for j in range(NW):
                fillers = []
                ws = [1, 2] if j == 0 else ([j + 2] if j + 2 < NW else [])
                for w in ws:
                    if w + 2 < NW:
                        fillers.append(f_xw_dma(w + 2))
                    for rb in range(3):
                        fillers.append(f_qkv_rb(w, rb))
                    for k in range(4):
                        fillers.append(f_vtrans(w, k))
                if j >= 2:
                    for ot in range(8):
                        fillers.append(f_proj(j - 2, ot))
                if j == NW - 1:
                    # window NW-1 is long and has no QKV left; its own
                    # tail(NW-2) resolves early, so proj(NW-2) fits here too
                    for ot in range(8):
                        fillers.append(f_proj(j - 1, ot))
                prev_tail = emit_attn(j, fillers, prev_tail, head_done=True)
            # keep the PE clock warm through the final tail's normalize
            # latency (the dummies have no deps, so they run right after the
            # last AV while the tail chain flows on DVE), then project
            pe_keepwarm(20)
            prev_tail()
            for ot in range(8):
                f_proj(NW - 1, ot, pool=ps_sc, tag="sc")()

    nc.compile()
    return nc


def make_inputs(x, W_qkv, b_qkv, W_proj):
    """Host-side shard prep. Returns in_maps for the 8 cores."""
    s = 1.0 / math.sqrt(64.0)
    xT = np.ascontiguousarray(x.T).astype(ml_dtypes.bfloat16)

    in_maps = []
    for i in range(NCORES):
        sl = slice(128 * i, 128 * i + 128)
        wshard = np.concatenate(
            [W_qkv[0:1024][sl] * s, W_qkv[1024:2048][sl], W_qkv[2048:3072][sl]], axis=0
        )  # [384, 1024]
        wqkvT = np.ascontiguousarray(wshard.T).astype(ml_dtypes.bfloat16)
        bq = np.stack(
            [b_qkv[0:1024][sl] * s, b_qkv[1024:2048][sl], b_qkv[2048:3072][sl]], axis=1
        ).astype(np.float32)  # [128, 3]
        wpT = np.ascontiguousarray(W_proj[:, sl].T).astype(ml_dtypes.bfloat16)
        in_maps.append({"xT": xT, "wqkvT": wqkvT, "bqkv": bq, "wpT": wpT})
    return in_maps


_NC_CACHE = {}


def get_nc():
    if "nc" not in _NC_CACHE:
        _NC_CACHE["nc"] = build_nc()
    return _NC_CACHE["nc"]


def kernel(x, W_qkv, b_qkv, W_proj, b_proj):
    x = np.asarray(x, dtype=np.float32)
    W_qkv = np.asarray(W_qkv, dtype=np.float32)
    b_qkv = np.asarray(b_qkv, dtype=np.float32)
    W_proj = np.asarray(W_proj, dtype=np.float32)
    b_proj = np.asarray(b_proj, dtype=np.float32)

    nc = get_nc()
    in_maps = make_inputs(x, W_qkv, b_qkv, W_proj)
    res = run_bass_kernel_spmd(nc, in_maps, core_ids=list(range(NCORES)))
    acc = np.zeros((DM, T), dtype=np.float64)
    for i in range(NCORES):
        acc += res.results[i]["out"].astype(np.float64)
    return (acc.T + b_proj[None, :].astype(np.float64)).astype(np.float32)


if __name__ == "__main__":
    rng = np.random.default_rng(0)
    x = rng.standard_normal((T, DM), dtype=np.float32)
    W_qkv = (rng.standard_normal((3 * DM, DM), dtype=np.float32) / 32.0).astype(
        np.float32
    )
    b_qkv = np.zeros((3 * DM,), dtype=np.float32)
    W_proj = (rng.standard_normal((DM, DM), dtype=np.float32) / 32.0).astype(np.float32)
    b_proj = np.zeros((DM,), dtype=np.float32)
    y = kernel(x, W_qkv, b_qkv, W_proj, b_proj)
    print("kernel output", y.shape, y.dtype)
